# revision 39
# baseline (speedup 1.0000x reference)
"""ChebNet (K=2) graph classifier on 8 Trainium2 NeuronCores.

Strategy (graph/data parallel, zero halo):
  - The 50 batched graphs are independent, so graphs are assigned whole to
    cores (7-graph slots; 50 = 2*7 + 6*6).  One SPMD program runs on all 8
    cores; cores with fewer graphs chew zero blocks.
  - The normalized aggregation Tx1 = -D^-1/2 A D^-1/2 feat is a dense
    per-graph matmul against the edge-count matrix C (structural, built
    host-side).  C is stored as EXACT fp8e4 small-int counts and streamed
    from HBM ONCE per graph, resident in SBUF across both Chebyshev layers
    (the baseline streamed scaled fp16 blocks twice: 4x the HBM traffic).
  - The degree scalings are factored out of C:  agg = C^T (dinv*feat),
    Tx1 = -dinv[dst] * agg.  The src scale rides the node-major stationary
    tiles (host-prescaled fp8 for layer 1, a fused DVE tensor_scalar after
    the on-chip transposes for layer 2); the dst scale is a host-staged
    -dinv broadcast tile multiplied into the PSUM->SBUF copy
    (scalar_tensor_tensor), replacing the plain copy at zero extra cost.
  - With both aggregation operands in fp8, the matmuls run in DoubleRow
    perf mode (256-deep contraction per pass) at free-dim 512/464, ~1.4-2x
    the fp16 PE rate.  Dense Chebyshev layers, bias+relu, max-pool readout
    and the classifier stay fp16/fp32 on-device.
"""

import sys

if "/opt/trn_rl_repo" not in sys.path:
    sys.path.insert(0, "/opt/trn_rl_repo")

import numpy as np
import ml_dtypes

# ---------------------------------------------------------------- constants
N = 100_000
E = 1_600_000
B = 50
GSIZE = 2000
D = 128  # IN == HID == 128
NCOUT = 10
NCORES = 8
NG = 7  # graph slots per core (50 = 2*7 + 6*6)
NWIN = 16  # src windows of 128
GSTRIDE = NWIN * 128  # 2048
QUADS = [(0, 512), (512, 512), (1024, 512), (1536, 464)]  # dst tiling of 2000
SROW = NWIN * GSIZE  # S cols per slot (quad-major: [q][t][qn])

F8 = ml_dtypes.float8_e4m3


# ---------------------------------------------------------------- host prep
def _preprocess(src, dst):
    """Structural preprocessing: graph->core assignment, degrees, and
    per-graph edge-count blocks [128, 16, 2000] (partition-major windows)."""
    deg = np.bincount(dst, minlength=N)
    dinv = (np.clip(deg.astype(np.float64), 1.0, None) ** -0.5).astype(np.float32)

    order = [0, 2, 1, 3, 4, 5, 6, 7]  # extra graphs land on cores 0 and 2
    slots = [[] for _ in range(NCORES)]
    for g in range(B):
        slots[order[g % NCORES]].append(g)

    g_of_e = dst // GSIZE
    flat = (src - g_of_e * GSIZE) * np.int64(GSIZE) + (dst - g_of_e * GSIZE)
    cblks = []
    for g in range(B):
        m = g_of_e == g
        cnt = np.bincount(flat[m], minlength=GSTRIDE * GSIZE).astype(np.float32)
        # [2048 src, 2000 dst] -> [128 p, 16 t, 2000 d]
        c = cnt.reshape(NWIN, 128, GSIZE).transpose(1, 0, 2)
        cblks.append(c.astype(F8))
    return dict(slots=slots, cblks=cblks, dinv=dinv)


# ---------------------------------------------------------------- program
def _build_program():
    from concourse import bacc, mybir, tile

    f8 = mybir.dt.float8e4
    f16 = mybir.dt.float16
    f32 = mybir.dt.float32
    AL = mybir.AluOpType
    DR = mybir.MatmulPerfMode.DoubleRow

    nc = bacc.Bacc(None, target_bir_lowering=False)

    xg_in = nc.declare_dram_parameter("XG", [128, NG * GSIZE], f16, isOutput=False)
    ynm_in = nc.declare_dram_parameter("YNM8", [128, NG * GSTRIDE], f8, isOutput=False)
    sc_in = nc.declare_dram_parameter("SC8", [128, NG * SROW], f8, isOutput=False)
    ndb_in = nc.declare_dram_parameter("NDB", [128, NG * GSIZE], f8, isOutput=False)
    dsb_in = nc.declare_dram_parameter(
        "DSRCB", [128, NG * GSTRIDE], f8, isOutput=False
    )
    w1a_in = nc.declare_dram_parameter("W1A", [128, 128], f16, isOutput=False)
    w1b_in = nc.declare_dram_parameter("W1B", [128, 128], f16, isOutput=False)
    w2a_in = nc.declare_dram_parameter("W2A", [128, 128], f16, isOutput=False)
    w2b_in = nc.declare_dram_parameter("W2B", [128, 128], f16, isOutput=False)
    b1_in = nc.declare_dram_parameter("B1", [128, 1], f32, isOutput=False)
    b2_in = nc.declare_dram_parameter("B2", [128, 1], f32, isOutput=False)
    wc_in = nc.declare_dram_parameter("WC", [128, NCOUT], f16, isOutput=False)
    bc_in = nc.declare_dram_parameter("BC", [1, NCOUT], f16, isOutput=False)
    ones_in = nc.declare_dram_parameter("ONES1", [1, NG], f16, isOutput=False)
    id_in = nc.declare_dram_parameter("IDENT", [128, 128], f16, isOutput=False)
    out_dram = nc.declare_dram_parameter("OUT", [NG, NCOUT], f32, isOutput=True)

    with tile.TileContext(nc) as tc:
        with (
            tc.tile_pool(name="const", bufs=1) as cpool,
            tc.tile_pool(name="sblk", bufs=3) as sbpool,
            tc.tile_pool(name="gin", bufs=3) as ginpool,
            tc.tile_pool(name="stg", bufs=2) as stgpool,
            tc.tile_pool(name="tx1", bufs=2) as tx1pool,
            tc.tile_pool(name="h1", bufs=2) as h1pool,
            tc.tile_pool(name="h2", bufs=2) as h2pool,
            tc.tile_pool(name="ptr", bufs=2, space="PSUM") as ptrpool,
            tc.tile_pool(name="pwin", bufs=3, space="PSUM") as pwinpool,
            tc.tile_pool(name="pd", bufs=3, space="PSUM") as pdpool,
        ):
            ident = cpool.tile([128, 128], f16, tag="ident")
            w1a = cpool.tile([128, 128], f16, tag="w1a")
            w1b = cpool.tile([128, 128], f16, tag="w1b")
            w2a = cpool.tile([128, 128], f16, tag="w2a")
            w2b = cpool.tile([128, 128], f16, tag="w2b")
            b1t = cpool.tile([128, 1], f32, tag="b1")
            b2t = cpool.tile([128, 1], f32, tag="b2")
            wct = cpool.tile([128, NCOUT], f16, tag="wc")
            bct = cpool.tile([1, NCOUT], f16, tag="bc")
            ones1 = cpool.tile([1, NG], f16, tag="ones1")
            hg = cpool.tile([128, NG], f16, tag="hg")
            outs = cpool.tile([NG, NCOUT], f32, tag="outs")

            def load_consts():
                nc.sync.dma_start(out=w1a[:], in_=w1a_in[:])
                nc.sync.dma_start(out=w1b[:], in_=w1b_in[:])
                nc.sync.dma_start(out=b1t[:], in_=b1_in[:])
                nc.sync.dma_start(out=ident[:], in_=id_in[:])
                nc.sync.dma_start(out=w2a[:], in_=w2a_in[:])
                nc.sync.dma_start(out=w2b[:], in_=w2b_in[:])
                nc.sync.dma_start(out=b2t[:], in_=b2_in[:])
                nc.sync.dma_start(out=wct[:], in_=wc_in[:])
                nc.sync.dma_start(out=bct[:], in_=bc_in[:])
                nc.sync.dma_start(out=ones1[:], in_=ones_in[:])

            for s in range(NG):
                # Per-graph inputs, loaded just-in-time (double-buffered so
                # graph s+1 streams while s computes).  DMA triggers cost
                # ~0.7us each on their issuing queue (packets then spread
                # across all 16 DMA engines regardless of issuer), so the
                # agg-critical stream (ynm + S) is triggered from the
                # otherwise-idle gpsimd queue and the rest from sync.
                ynm = ginpool.tile([128, NWIN, 128], f8, tag="ynm")
                nc.gpsimd.dma_start(
                    out=ynm[:],
                    in_=ynm_in[:, s * GSTRIDE : (s + 1) * GSTRIDE].rearrange(
                        "p (t f) -> p t f", f=128
                    ),
                )
                sbq = []
                for qi, (qoff, qn) in enumerate(QUADS):
                    sb = sbpool.tile([128, NWIN, qn], f8, tag=f"sb{qi}")
                    c0 = s * SROW + qoff * NWIN
                    half = (NWIN // 2) * qn
                    for hh in range(2):
                        nc.gpsimd.dma_start(
                            out=sb[:, hh * (NWIN // 2) : (hh + 1) * (NWIN // 2), :],
                            in_=sc_in[
                                :, c0 + hh * half : c0 + (hh + 1) * half
                            ].rearrange("p (t d) -> p t d", t=NWIN // 2),
                        )
                    sbq.append(sb)
                ndb = ginpool.tile([128, GSIZE], f8, tag="ndb")
                xg = ginpool.tile([128, GSIZE], f16, tag="xg")
                dsrcb = ginpool.tile([128, GSTRIDE], f8, tag="dsrcb")
                nc.sync.dma_start(
                    out=ndb[:], in_=ndb_in[:, s * GSIZE : (s + 1) * GSIZE]
                )
                nc.sync.dma_start(
                    out=xg[:], in_=xg_in[:, s * GSIZE : (s + 1) * GSIZE]
                )
                nc.sync.dma_start(
                    out=dsrcb[:],
                    in_=dsb_in[:, s * GSTRIDE : (s + 1) * GSTRIDE],
                )
                if s == 0:
                    # weight consts are needed later than graph 0's inputs;
                    # keep them behind the startup-critical triggers
                    load_consts()

                h1 = h1pool.tile([128, GSTRIDE], f16, tag="h1")
                h2 = h2pool.tile([128, GSIZE], f16, tag="h2")
                hq4 = ginpool.tile([128, 4], f16, tag="hq4")

                for layer in range(2):
                    if layer == 0:
                        stg3 = ynm
                    else:
                        # node-major dinv-scaled fp8 copy of h1: PE
                        # transposes batched 4-per-PSUM-bank (one matmul
                        # group, disjoint column ranges), then one fused
                        # DVE scale/cast per batch
                        stg2 = stgpool.tile([128, NWIN, 128], f8, tag="stg2")
                        for j in range(NWIN // 4):
                            ptr4 = ptrpool.tile([128, 512], f32, tag="ptr")
                            for k in range(4):
                                t = j * 4 + k
                                nc.tensor.matmul(
                                    ptr4[:, k * 128 : (k + 1) * 128],
                                    h1[:, t * 128 : (t + 1) * 128],
                                    ident[:],
                                    start=(k == 0),
                                    stop=(k == 3),
                                    skip_group_check=True,
                                )
                            nc.vector.scalar_tensor_tensor(
                                stg2[:, j * 4 : (j + 1) * 4, :],
                                ptr4[:],
                                1.0,
                                dsrcb[:, j * 512 : (j + 1) * 512],
                                AL.mult,
                                AL.mult,
                            )
                        stg3 = stg2

                    # aggregation: tx1[f, d] = -dinv[d] * sum_s y[s, f] C[s, d]
                    # quad-outer: quad 0 can start as soon as its S DMA lands
                    tx1 = tx1pool.tile([128, GSIZE], f16, tag="tx1")
                    for qi, (qoff, qn) in enumerate(QUADS):
                        pwin = pwinpool.tile([128, 512], f32, tag="pwin")
                        for th in range(NWIN // 2):
                            nc.tensor.matmul(
                                pwin[:, :qn],
                                stg3[:, 2 * th : 2 * th + 2, :],
                                sbq[qi][:, 2 * th : 2 * th + 2, :],
                                start=(th == 0),
                                stop=(th == NWIN // 2 - 1),
                                perf_mode=DR,
                            )
                        nc.vector.scalar_tensor_tensor(
                            tx1[:, qoff : qoff + qn],
                            pwin[:, :qn],
                            1.0,
                            ndb[:, qoff : qoff + qn],
                            AL.mult,
                            AL.mult,
                        )

                    # dense: h = relu([Tx0, Tx1] @ W + b); bias+relu on the
                    # (otherwise idle) scalar engine
                    wa, wb = (w1a, w1b) if layer == 0 else (w2a, w2b)
                    bt = b1t if layer == 0 else b2t
                    for qoff, qn in QUADS:
                        pd = pdpool.tile([128, 512], f32, tag="pd")
                        rhs0 = (
                            xg[:, qoff : qoff + qn]
                            if layer == 0
                            else h1[:, qoff : qoff + qn]
                        )
                        nc.tensor.matmul(
                            pd[:, :qn], wa[:], rhs0, start=True, stop=False
                        )
                        nc.tensor.matmul(
                            pd[:, :qn],
                            wb[:],
                            tx1[:, qoff : qoff + qn],
                            start=False,
                            stop=True,
                        )
                        dst_ap = (
                            h1[:, qoff : qoff + qn]
                            if layer == 0
                            else h2[:, qoff : qoff + qn]
                        )
                        nc.scalar.activation(
                            dst_ap,
                            pd[:, :qn],
                            mybir.ActivationFunctionType.Relu,
                            bias=bt[:],
                            scale=1.0,
                        )
                    if layer == 0:
                        nc.vector.memset(h1[:, GSIZE:GSTRIDE], 0.0)
                    else:
                        # per-quad partial max (chains with the act drain),
                        # then a tiny final max
                        for qi, (qoff, qn) in enumerate(QUADS):
                            nc.vector.tensor_reduce(
                                hq4[:, qi : qi + 1],
                                h2[:, qoff : qoff + qn],
                                mybir.AxisListType.X,
                                AL.max,
                            )
                        nc.vector.tensor_reduce(
                            hg[:, s : s + 1],
                            hq4[:],
                            mybir.AxisListType.X,
                            AL.max,
                        )

            # ---- readout: out = HG^T @ Wc + 1^T @ bc
            pot = pdpool.tile([128, 512], f32, tag="pd")
            po = pot[:NG, :NCOUT]
            nc.tensor.matmul(po, hg[:, :NG], wct[:], start=True, stop=False)
            nc.tensor.matmul(po, ones1[:], bct[:], start=False, stop=True)
            nc.vector.tensor_copy(outs[:], po)
            nc.sync.dma_start(out=out_dram[:], in_=outs[:])

    nc.compile()
    return nc


# ---------------------------------------------------------------- host glue
def _make_core_inputs(x, W1, b1, W2, b2, Wc, bc, pre):
    dinv = pre["dinv"]
    in_maps = []
    for c in range(NCORES):
        xg = np.zeros((128, NG * GSIZE), dtype=np.float16)
        ynm = np.zeros((128, NG * GSTRIDE), dtype=F8)
        sc = np.zeros((128, NG * SROW), dtype=F8)
        ndb = np.zeros((128, NG * GSIZE), dtype=F8)
        dsrcb = np.zeros((128, NG * GSTRIDE), dtype=F8)
        for s, g in enumerate(pre["slots"][c]):
            xgf = x[g * GSIZE : (g + 1) * GSIZE]  # [2000, 128] f32
            dv = dinv[g * GSIZE : (g + 1) * GSIZE]  # [2000]
            xg[:, s * GSIZE : (s + 1) * GSIZE] = xgf.T.astype(np.float16)
            ndb[:, s * GSIZE : (s + 1) * GSIZE] = np.broadcast_to(
                (-dv).astype(F8), (128, GSIZE)
            )
            y = np.zeros((GSTRIDE, 128), dtype=np.float32)
            y[:GSIZE] = dv[:, None] * xgf
            # [2048, 128] -> [128 p, 16 t, 128 f]
            ynm[:, s * GSTRIDE : (s + 1) * GSTRIDE] = (
                y.reshape(NWIN, 128, 128).transpose(1, 0, 2).reshape(128, GSTRIDE)
            ).astype(F8)
            dvp = np.zeros(GSTRIDE, dtype=np.float32)
            dvp[:GSIZE] = dv
            # dsrcb[p, t*128 + f] = dinv[t*128 + p] (node-major, bcast over f)
            dsrcb[:, s * GSTRIDE : (s + 1) * GSTRIDE] = np.broadcast_to(
                dvp.reshape(NWIN, 128).T.astype(F8)[:, :, None],
                (128, NWIN, 128),
            ).reshape(128, GSTRIDE)
            cb = pre["cblks"][g]  # [128, 16, 2000] f8
            parts = [
                cb[:, :, qoff : qoff + qn].reshape(128, NWIN * qn)
                for qoff, qn in QUADS
            ]
            sc[:, s * SROW : (s + 1) * SROW] = np.concatenate(parts, axis=1)
        in_maps.append(
            dict(
                XG=xg,
                YNM8=ynm,
                SC8=sc,
                NDB=ndb,
                DSRCB=dsrcb,
                W1A=np.ascontiguousarray(W1[:128]).astype(np.float16),
                W1B=np.ascontiguousarray(W1[128:]).astype(np.float16),
                W2A=np.ascontiguousarray(W2[:128]).astype(np.float16),
                W2B=np.ascontiguousarray(W2[128:]).astype(np.float16),
                B1=b1.reshape(128, 1).astype(np.float32),
                B2=b2.reshape(128, 1).astype(np.float32),
                WC=Wc.astype(np.float16),
                BC=bc.reshape(1, NCOUT).astype(np.float16),
                ONES1=np.ones((1, NG), dtype=np.float16),
                IDENT=np.eye(128, dtype=np.float16),
            )
        )
    return in_maps


_CACHE = {}


def kernel(x, W1, b1, W2, b2, Wc, bc, src, dst, graph_ids, _trace=False):
    from concourse.bass_utils import run_bass_kernel_spmd

    x = np.asarray(x, dtype=np.float32)
    src = np.asarray(src).astype(np.int64)
    dst = np.asarray(dst).astype(np.int64)

    pre = _preprocess(src, dst)
    if "prog" not in _CACHE:
        _CACHE["prog"] = _build_program()
    nc = _CACHE["prog"]

    in_maps = _make_core_inputs(
        x,
        np.asarray(W1, np.float32),
        np.asarray(b1, np.float32),
        np.asarray(W2, np.float32),
        np.asarray(b2, np.float32),
        np.asarray(Wc, np.float32),
        np.asarray(bc, np.float32),
        pre,
    )
    res = run_bass_kernel_spmd(nc, in_maps, list(range(NCORES)), trace=_trace)

    out = np.zeros((B, NCOUT), dtype=np.float32)
    for c in range(NCORES):
        oc = res.results[c]["OUT"]
        for s, g in enumerate(pre["slots"][c]):
            out[g] = oc[s]
    if _trace:
        kernel._last_exec_ns = res.exec_time_ns
    return out
